# revision 4
# baseline (speedup 1.0000x reference)
"""Trainium2 Bass kernel for nn_AutoEncoderTucker.

Math (reference):
    A   = X @ kron(C_inv, B_inv).T @ G_inv            (encode,  N x R1)
    out = softmax(A) @ relu(G) @ kron(sm(C), sm(B)).T (decode,  N x J*K)

Instead of materializing the (36864 x 2304) Kronecker factors (~700 GFLOP),
the kernel exploits the Tucker factorization:
    encode:  Z[(k,r2), n] = sum_j B_inv[r2,j] X[n,(k,j)]         (per-k matmul)
             A = Z.T @ M1,  M1[(k,r2),r1] = sum_r3 C_inv[r3,k] G_inv[(r3,r2),r1]
    decode:  W[(k,r2), n] = M2.T @ smA.T, M2[r1,(k,r2)] = sum_r3 smC[k,r3] reluG[r1,(r3,r2)]
             out[n,(k,j)] = W_k.T @ smB.T
M1 / M2 are tiny-host-precomputed (fold the r3 contraction); r2 is padded
48->64 so two k's pack into one 128-partition tile.

Sharding: pure data-parallel over N across the 8 cores (256 rows each);
all small matrices replicated. No collectives.
"""
import numpy as np

# ---- problem shapes (hardcoded; kernel.py must be self-contained) ----
N, JK = 2048, 36864
J = K = 192
R1, R2, R3 = 256, 48, 48
NCORES = 8
NSH = N // NCORES          # 256 samples per core
R2P = 64                   # r2 padded
T = 96                     # k-pair tiles (2 k's each, 128 padded (k,r2) rows)
NG = 12                    # groups of t
TPG = T // NG              # 8 t per group
FPG = TPG * 2 * J          # 3072 features (and out columns) per group
CKP = K * R2P              # 12288 padded (k,r2) flat size

_CACHE: dict = {}


def _softmax64(t):
    e = np.exp(t - t.max(axis=-1, keepdims=True))
    return e / e.sum(axis=-1, keepdims=True)


def host_precompute(B, C, Gm, B_inv, C_inv, G_inv):
    f64 = np.float64
    B, C, Gm = np.asarray(B, f64), np.asarray(C, f64), np.asarray(Gm, f64)
    B_inv = np.asarray(B_inv, f64)
    C_inv = np.asarray(C_inv, f64)
    G_inv = np.asarray(G_inv, f64)

    smB, smC = _softmax64(B), _softmax64(C)
    reluG = np.maximum(Gm, 0.0)

    # M1[k*64+r2, r1] = sum_r3 C_inv[r3,k] * G_inv[r3*48+r2, r1]
    G3 = G_inv.reshape(R3, R2, R1)
    M1c = np.einsum('rk,rsp->ksp', C_inv, G3)
    M1p = np.zeros((K, R2P, R1), f64)
    M1p[:, :R2, :] = M1c
    M1p = np.ascontiguousarray(M1p.reshape(CKP, R1), np.float32)

    # M2[r1, k*64+r2] = sum_r3 smC[k,r3] * reluG[r1, r3*48+r2]
    G3d = reluG.reshape(R1, R3, R2)
    M2c = np.einsum('kr,prs->pks', smC, G3d)
    M2p = np.zeros((R1, K, R2P), f64)
    M2p[:, :, :R2] = M2c
    M2p = np.ascontiguousarray(M2p.reshape(R1, CKP), np.float32)

    # S1 weights: B_inv.T (j, r2) zero-padded to 64 cols, packed for the
    # 3-tile / 2-k X layout.  w1 = [Wab | Wcd] (128, 256)
    BinvTp = np.zeros((J, R2P), f64)
    BinvTp[:, :R2] = B_inv.T
    w1 = np.zeros((128, 256), f64)
    w1[0:128, 0:64] = BinvTp[0:128]        # k-even, contract j 0:128   @ base 0
    w1[0:64, 64:128] = BinvTp[128:192]     # k-even, contract j 128:192 @ base 0
    w1[64:128, 128:192] = BinvTp[0:64]     # k-odd,  contract j 0:64    @ base 64
    w1[0:128, 192:256] = BinvTp[64:192]    # k-odd,  contract j 64:192  @ base 0
    w1 = np.ascontiguousarray(w1, np.float32)

    # smB.T duplicated at partition bases 0 and 64 (to match lhsT bases)
    smbt2 = np.zeros((128, J), f64)
    smbt2[0:R2] = smB.T
    smbt2[64:64 + R2] = smB.T
    smbt2 = np.ascontiguousarray(smbt2, np.float32)

    ident = np.eye(128, dtype=np.float32)
    return {"w1": w1, "m1p": M1p, "m2p": M2p, "smbt2": smbt2, "ident": ident}


def build_nc(n_groups=NG):
    """Build + bacc-compile the per-core Tile kernel."""
    import concourse.bass as bass
    import concourse.bacc as bacc
    import concourse.mybir as mybir
    from concourse import tile

    f32 = mybir.dt.float32
    PS = bass.MemorySpace.PSUM
    AX = mybir.AxisListType.X
    AF = mybir.ActivationFunctionType
    ngf = n_groups * FPG
    nt = n_groups * TPG

    nc = bacc.Bacc(None, target_bir_lowering=False, debug=False,
                   num_devices=NCORES)

    x = nc.dram_tensor("x", [NSH, JK], f32, kind="ExternalInput")
    m1p = nc.dram_tensor("m1p", [CKP, R1], f32, kind="ExternalInput")
    m2p = nc.dram_tensor("m2p", [R1, CKP], f32, kind="ExternalInput")
    w1 = nc.dram_tensor("w1", [128, 256], f32, kind="ExternalInput")
    smbt2 = nc.dram_tensor("smbt2", [128, J], f32, kind="ExternalInput")
    ident = nc.dram_tensor("ident", [128, 128], f32, kind="ExternalInput")
    out = nc.dram_tensor("out", [NSH, JK], f32, kind="ExternalOutput")

    with tile.TileContext(nc) as tc:
        with tc.tile_pool(name="const", bufs=1) as cpool:
            w1_t = cpool.tile([128, 256], f32, tag="w1", name="w1")
            nc.sync.dma_start(w1_t[:], w1[:])
            smbt2_t = cpool.tile([128, J], f32, tag="smbt2", name="smbt2")
            nc.sync.dma_start(smbt2_t[:], smbt2[:])
            ident_t = cpool.tile([128, 128], f32, tag="ident", name="ident")
            nc.sync.dma_start(ident_t[:], ident[:])
            # smA.T halves: (r1-half, n=256), persistent across phases
            smat = [cpool.tile([128, 256], f32, tag=f"smat{h}", name=f"smat{h}") for h in range(2)]

            # ---------------- phase 1: encode ----------------
            with (
                tc.tile_pool(name="xrow", bufs=2) as xrow_pool,
                tc.tile_pool(name="xt", bufs=4) as xt_pool,
                tc.tile_pool(name="ztsb", bufs=3) as zt_pool,
                tc.tile_pool(name="m1", bufs=2) as m1_pool,
                tc.tile_pool(name="smx", bufs=1) as smx_pool,
                tc.tile_pool(name="tp_ps", bufs=2, space=PS) as tp_ps,
                tc.tile_pool(name="zt_ps", bufs=2, space=PS) as zt_ps,
                tc.tile_pool(name="a_ps", bufs=1, space=PS) as a_ps,
            ):
                a_psum = [a_ps.tile([128, R1], f32, tag=f"a{nb}", name=f"a{nb}") for nb in range(2)]
                for g in range(n_groups):
                    xr = []
                    for nb in range(2):
                        t_ = xrow_pool.tile([128, FPG], f32, tag=f"xr{nb}", name=f"xr{nb}")
                        nc.sync.dma_start(
                            t_[:], x[nb * 128:(nb + 1) * 128, g * FPG:(g + 1) * FPG])
                        xr.append(t_)
                    m1_t = m1_pool.tile([128, TPG, R1], f32, tag="m1", name="m1")
                    nc.sync.dma_start(
                        m1_t[:],
                        m1p[g * TPG * 128:(g + 1) * TPG * 128, :]
                        .rearrange("(i p) r -> p i r", p=128))
                    for i in range(TPG):
                        t = g * TPG + i
                        xt = []
                        for c in range(3):
                            xt_t = xt_pool.tile([128, 256], f32, tag="xt", name="xt")
                            for nb in range(2):
                                tp = tp_ps.tile([128, 128], f32, tag="tp", name="tp")
                                nc.tensor.transpose(
                                    tp[:],
                                    xr[nb][:, (i * 3 + c) * 128:(i * 3 + c + 1) * 128],
                                    ident_t[:])
                                nc.vector.tensor_copy(
                                    xt_t[:, nb * 128:(nb + 1) * 128], tp[:])
                            xt.append(xt_t)
                        ztp = zt_ps.tile([128, 256], f32, tag="ztp", name="ztp")
                        nc.tensor.matmul(ztp[0:64, :], w1_t[0:128, 0:64],
                                         xt[0][:], start=True, stop=False)
                        nc.tensor.matmul(ztp[0:64, :], w1_t[0:64, 64:128],
                                         xt[1][0:64, :], start=False, stop=True)
                        nc.tensor.matmul(ztp[64:128, :], w1_t[64:128, 128:192],
                                         xt[1][64:128, :], start=True, stop=False)
                        nc.tensor.matmul(ztp[64:128, :], w1_t[0:128, 192:256],
                                         xt[2][:], start=False, stop=True)
                        zts = zt_pool.tile([128, 256], f32, tag="zt", name="zt")
                        nc.scalar.copy(zts[:], ztp[:])
                        for nb in range(2):
                            nc.tensor.matmul(
                                a_psum[nb][:],
                                zts[:, nb * 128:(nb + 1) * 128],
                                m1_t[:, i, :],
                                start=(t == 0), stop=(t == nt - 1))
                # softmax along r1 (free dim) + PE transpose into smat
                for nb in range(2):
                    nmax = smx_pool.tile([128, 1], f32, tag=f"nmax{nb}", name=f"nmax{nb}")
                    nc.vector.reduce_max(nmax[:], a_psum[nb][:], axis=AX, negate=True)
                    esum = smx_pool.tile([128, 1], f32, tag=f"esum{nb}", name=f"esum{nb}")
                    expt = smx_pool.tile([128, 256], f32, tag=f"expt{nb}", name=f"expt{nb}")
                    nc.scalar.activation(expt[:], a_psum[nb][:], AF.Exp,
                                         bias=nmax[:], scale=1.0,
                                         accum_out=esum[:])
                    rinv = smx_pool.tile([128, 1], f32, tag=f"rinv{nb}", name=f"rinv{nb}")
                    nc.vector.reciprocal(rinv[:], esum[:])
                    sma = smx_pool.tile([128, 256], f32, tag=f"sma{nb}", name=f"sma{nb}")
                    nc.vector.tensor_scalar_mul(sma[:], expt[:], rinv[:])
                    for h in range(2):
                        tp = tp_ps.tile([128, 128], f32, tag="tp", name="tp")
                        nc.tensor.transpose(
                            tp[:], sma[:, h * 128:(h + 1) * 128], ident_t[:])
                        nc.vector.tensor_copy(
                            smat[h][:, nb * 128:(nb + 1) * 128], tp[:])

            # ---------------- phase 2: decode ----------------
            with (
                tc.tile_pool(name="m2", bufs=2) as m2_pool,
                tc.tile_pool(name="wtsb", bufs=3) as wt_pool,
                tc.tile_pool(name="osb", bufs=2) as osb_pool,
                tc.tile_pool(name="wt_ps", bufs=2, space=PS) as wt_ps,
                tc.tile_pool(name="o_ps", bufs=4, space=PS) as o_ps,
            ):
                for g in range(n_groups):
                    m2_t = []
                    for h in range(2):
                        t_ = m2_pool.tile([128, TPG * 128], f32, tag=f"m2h{h}", name=f"m2h{h}")
                        nc.sync.dma_start(
                            t_[:],
                            m2p[h * 128:(h + 1) * 128,
                                g * TPG * 128:(g + 1) * TPG * 128])
                        m2_t.append(t_)
                    osb = [osb_pool.tile([128, FPG], f32, tag=f"osb{nb}", name=f"osb{nb}")
                           for nb in range(2)]
                    for i in range(TPG):
                        wtp = wt_ps.tile([128, 256], f32, tag="wtp", name="wtp")
                        for h in range(2):
                            nc.tensor.matmul(wtp[:],
                                             m2_t[h][:, i * 128:(i + 1) * 128],
                                             smat[h][:],
                                             start=(h == 0), stop=(h == 1))
                        wts = wt_pool.tile([128, 256], f32, tag="wt", name="wt")
                        nc.scalar.copy(wts[:], wtp[:])
                        for kk in range(2):
                            base = kk * 64
                            for nb in range(2):
                                op = o_ps.tile([128, J], f32, tag="op", name="op")
                                nc.tensor.matmul(
                                    op[:],
                                    wts[base:base + R2, nb * 128:(nb + 1) * 128],
                                    smbt2_t[base:base + R2, :])
                                eng = nc.vector if (kk + nb) % 2 == 0 else nc.scalar
                                if eng is nc.vector:
                                    eng.tensor_copy(
                                        osb[nb][:, (i * 2 + kk) * J:(i * 2 + kk + 1) * J],
                                        op[:])
                                else:
                                    eng.copy(
                                        osb[nb][:, (i * 2 + kk) * J:(i * 2 + kk + 1) * J],
                                        op[:])
                    for nb in range(2):
                        nc.sync.dma_start(
                            out[nb * 128:(nb + 1) * 128, g * FPG:(g + 1) * FPG],
                            osb[nb][:])
    nc.compile()
    return nc


def _get_nc(n_groups=NG):
    key = ("nc", n_groups)
    if key not in _CACHE:
        _CACHE[key] = build_nc(n_groups)
    return _CACHE[key]


def make_in_maps(X, consts):
    X = np.ascontiguousarray(np.asarray(X, np.float32))
    return [
        {"x": X[c * NSH:(c + 1) * NSH], **consts}
        for c in range(NCORES)
    ]


def run(inputs, trace=False, n_groups=NG, **kwargs):
    """Run on 8 cores; returns (full output, BassKernelResults)."""
    from concourse.bass_utils import run_bass_kernel_spmd
    consts = host_precompute(inputs["B"], inputs["C"], inputs["G"],
                             inputs["B_inv"], inputs["C_inv"], inputs["G_inv"])
    in_maps = make_in_maps(inputs["X"], consts)
    nc = _get_nc(n_groups)
    res = run_bass_kernel_spmd(nc, in_maps, core_ids=list(range(NCORES)),
                               trace=trace, **kwargs)
    outs = [res.results[c]["out"] for c in range(NCORES)]
    full = np.concatenate(outs, axis=0).astype(np.float32)
    return full, res


def kernel(X, B, C, G, B_inv, C_inv, G_inv):
    full, _ = run(dict(X=X, B=B, C=C, G=G,
                       B_inv=B_inv, C_inv=C_inv, G_inv=G_inv))
    return full
